# revision 1
# baseline (speedup 1.0000x reference)
"""Trainium2 Bass kernel for nn_Damping: MLP trunk -> huge output layer ->
tril scatter -> D = L @ L.T, distributed over 8 NeuronCores.

Strategy (tensor-parallel over the 131328-wide output layer):
  - Host: fold biases into augmented trunk weights; permute + pad Wo's columns
    into a "flipped column-major" layout so that the triangular scatter on
    device becomes a single dma_gather with 64-element-aligned windows.
  - Device (SPMD x8): trunk MLP replicated; each core streams its 1024x18432
    f32 Wo shard from HBM through PE matmuls (M=1, N=512, 8 K-chunks),
    producing an 18432-element slice of the permuted output vector o'.
    AllGather o' -> every core gathers the 512x512 matrix V = L'^T (L' = JLJ,
    J = flip) via dma_gather + masks, computes D' = V^T V with 16 matmuls,
    and writes D = J D' J via a flipped output DMA.

The math: L lower-triangular (diag = exp(o[:512]), strict-lower = o[512:] in
row-major tril order). With J the anti-identity, L' = J L J is upper
triangular and D = L L^T = J (L' L'^T) J.  Row k of V = L'^T is
  [ L[511, 511-k], L[510, 511-k], ..., L[512-k, 511-k], exp-diag(511-k), 0... ]
i.e. column (511-k) of L read bottom-up: its data starts at COLUMN 0, which is
what makes a fixed 512-wide gather window land the data in the right place.
"""

import sys

sys.path.insert(0, "/opt/trn_rl_repo")

import numpy as np

import concourse.bass as bass
import concourse.bacc as bacc
import concourse.mybir as mybir
import concourse.tile as tile
from concourse.ap import AP
from concourse import bass_utils

N = 512
HID = 1024
OUT = N + N * (N - 1) // 2  # 131328
NCORES = 8
KC = HID // 128  # 8 k-chunks of the 1024-dim contraction

F32 = mybir.dt.float32
I16 = mybir.dt.int16


def _seg_starts():
    """64-aligned start (in elements of o') of segment g, g=0..511.

    o'[0:512] holds the flipped diag; segment g (g>=1) holds the g
    strict-lower elements of L column (511-g), bottom-up, zero-padded to a
    multiple of 64 (the padding comes from zero columns of the permuted Wo).
    """
    starts = np.zeros(N, dtype=np.int64)
    pos = N
    for g in range(1, N):
        starts[g] = pos
        pos += 64 * ((g + 63) // 64)
    return starts, int(pos)


TSTART, OTOT = _seg_starts()  # OTOT == 147456
assert OTOT == 147456
OSH = OTOT // NCORES  # 18432 per-core o' shard
NT = OSH // 512  # 36 psum tiles per core
NWIN = OTOT // 64 - 8 + 1  # 2297 gather windows cover the buffer exactly


def _colmap():
    """colmap[t] = original Wo column (o element) feeding o'[t], or -1 (pad)."""
    cm = np.full(OTOT, -1, dtype=np.int64)
    t = np.arange(N)
    cm[0:N] = (N - 1) - t  # flipped diag: o'[t] = o[511-t]
    for g in range(1, N):
        i = np.arange(g)
        r = (N - 1) - i  # L row index, from 511 downward
        c = (N - 1) - g  # L col index
        cm[TSTART[g] + i] = N + r * (r - 1) // 2 + c
    return cm


COLMAP = _colmap()


def _gather_idx():
    """int16 [16, 32] wrapped index tile: window start / 64 per V row g."""
    idx = np.zeros(N, dtype=np.int64)
    idx[0] = N // 64  # row 0 has no off-diag data; any in-bounds window
    idx[1:] = TSTART[1:] // 64
    assert idx.max() < NWIN
    wrapped = np.zeros((16, N // 16), dtype=np.int16)
    for g in range(N):
        wrapped[g % 16, g // 16] = idx[g]
    # replicated across the 8 GPSIMD cores' 16-partition groups
    return np.tile(wrapped, (8, 1))


GIDX = _gather_idx()

_PROGRAM_CACHE = {}


def build_program(debug_taps=False, reps=1, stream_only=False, loop_n=1):
    key = ("nc", debug_taps, reps, stream_only, loop_n)
    if key in _PROGRAM_CACHE:
        return _PROGRAM_CACHE[key]

    nc = bacc.Bacc("TRN2", target_bir_lowering=False, debug=False,
                   num_devices=NCORES)

    x_d = nc.dram_tensor("x", [N], F32, kind="ExternalInput")
    w0_d = nc.dram_tensor("w0", [4 * 128 + 1, HID], F32, kind="ExternalInput")
    w1_d = nc.dram_tensor("w1", [KC * 128 + 1, HID], F32, kind="ExternalInput")
    w2_d = nc.dram_tensor("w2", [KC * 128 + 1, HID], F32, kind="ExternalInput")
    wo_d = nc.dram_tensor("wo", [HID, OSH], F32, kind="ExternalInput")
    wob_d = nc.dram_tensor("wob", [OSH], F32, kind="ExternalInput")
    gidx_d = nc.dram_tensor("gidx", list(GIDX.shape), I16, kind="ExternalInput")
    out_d = nc.dram_tensor("out", [N, N], F32, kind="ExternalOutput")
    if debug_taps:
        dbg_h2 = nc.dram_tensor("dbg_h2", [128, 9], F32, kind="ExternalOutput")
        dbg_of = nc.dram_tensor("dbg_ofull", [OTOT], F32, kind="ExternalOutput")
        dbg_lt = nc.dram_tensor("dbg_lt", [128, 4 * 512], F32,
                                kind="ExternalOutput")
        dbg_ltm = nc.dram_tensor("dbg_ltm", [128, 4 * 512], F32,
                                 kind="ExternalOutput")

    with tile.TileContext(nc) as tc:
        with (
            tc.tile_pool(name="wop", bufs=4) as wop,
            tc.tile_pool(name="trunkp", bufs=4) as trunkp,
            tc.tile_pool(name="persist", bufs=1) as persist,
            tc.tile_pool(name="stagep", bufs=2) as stagep,
            tc.tile_pool(name="psum", bufs=6, space="PSUM") as psum,
            tc.tile_pool(name="dram", bufs=2, space="DRAM") as dram,
        ):
          def _emit_body():
            # ---- static masks ------------------------------------------------
            # iota_t[p, j] = j - p; row-index of V chunk c at partition p is
            # g = 128c + p, so (j < g) <=> (iota < 128c), (j == g) <=> (== 128c)
            iota_t = persist.tile([128, 512], mybir.dt.int32, tag="iota")
            nc.gpsimd.iota(iota_t[:], pattern=[[1, 512]], base=0,
                           channel_multiplier=-1)
            ltm = []  # keep-mask: 1.0 where col < row-index (the off-diag data)
            eqm = []  # 1.0 where col == row-index (the diag position)
            for c in range(4):
                m = persist.tile([128, 512], F32, tag=f"ltm{c}")
                nc.vector.tensor_scalar(m[:], iota_t[:], 128 * c, None,
                                        mybir.AluOpType.is_lt)
                ltm.append(m)
                e = persist.tile([128, 512], F32, tag=f"eqm{c}")
                nc.vector.tensor_scalar(e[:], iota_t[:], 128 * c, None,
                                        mybir.AluOpType.is_equal)
                eqm.append(e)

            gidx_sb = persist.tile(list(GIDX.shape), I16, tag="gidx")
            nc.gpsimd.dma_start(gidx_sb[:], gidx_d[:])

            # ---- trunk: x -> h2 (all on partition-0 rows + kT transposes) ----
            def to_kT(src_ap_flat, n_elems, tag):
                """DRAM [n_elems] -> SBUF [128, n_elems//128 + 1] k-chunk
                layout with a trailing [1,0,..] column for the bias matmul."""
                ncols = n_elems // 128
                hk = persist.tile([128, ncols + 1], F32, tag=tag)
                nc.vector.memset(hk[:, ncols:ncols + 1], 0.0)
                nc.vector.memset(hk[0:1, ncols:ncols + 1], 1.0)
                # fine-strided (4B/partition) pattern: HWDGE wedges on it,
                # SWDGE (gpsimd) handles it
                nc.gpsimd.dma_start(
                    hk[:, 0:ncols],
                    AP(src_ap_flat.tensor, src_ap_flat.offset,
                       [[1, 128], [128, ncols]]),
                )
                return hk

            x_kT = to_kT(x_d.ap(), N, "xkT")

            def trunk_layer(h_kT, w_dram, kchunks, tag):
                wb = stagep.tile([1, HID], F32, tag="wbias")
                nc.sync.dma_start(wb[:], w_dram[kchunks * 128:kchunks * 128 + 1, :])
                h_sb = persist.tile([1, HID], F32, tag="h_sb")
                for nh in range(2):
                    ps = psum.tile([128, 512], F32, tag="ps")
                    for kc in range(kchunks):
                        wt = trunkp.tile([128, 512], F32, tag="wt")
                        nc.sync.dma_start(
                            wt[:],
                            w_dram[kc * 128:(kc + 1) * 128,
                                   nh * 512:(nh + 1) * 512],
                        )
                        nc.tensor.matmul(ps[0:1, :], h_kT[:, kc:kc + 1], wt[:],
                                         start=(kc == 0), stop=False)
                    nc.tensor.matmul(ps[0:1, :],
                                     h_kT[0:1, kchunks:kchunks + 1],
                                     wb[0:1, nh * 512:(nh + 1) * 512],
                                     start=False, stop=True)
                    nc.scalar.activation(h_sb[0:1, nh * 512:(nh + 1) * 512],
                                         ps[0:1, :],
                                         mybir.ActivationFunctionType.Tanh)
                # bounce through DRAM to re-layout [1,1024] -> [128, 8+1]
                hd = dram.tile([HID], F32, tag="hdram")
                nc.sync.dma_start(hd[:], h_sb[0:1, :])
                hdap = hd[:]
                return to_kT(hdap, HID, f"kT_{tag}")

            h0_kT = trunk_layer(x_kT, w0_d, 4, "l0")
            h1_kT = trunk_layer(h0_kT, w1_d, KC, "l1")
            h2_kT = trunk_layer(h1_kT, w2_d, KC, "l2")

            # ---- output layer: o'_shard = h2 @ Wo_shard + bo_shard -----------
            # Wo streamed as 4MB tiles (2 n-tiles of 512 each x 8 k-chunks),
            # alternating between the two HWDGE rings (sync / scalar) so the
            # per-DMA completion latency hides under the other ring's
            # transfer and the stream runs at HBM bandwidth.
            o_shard = dram.tile([OSH], F32, tag="oshard")
            for tt in range(NT // 2):
                eng = nc.sync if (tt % 2 == 0) else nc.scalar
                wt = wop.tile([128, 2 * KC * 512], F32, tag="wo")
                eng.dma_start(
                    wt[:],
                    AP(wo_d, tt * 1024,
                       [[OSH, 128], [128 * OSH, KC], [1, 1024]]),
                )
                for half in range(2):
                    t = 2 * tt + half
                    ps = psum.tile([128, 512], F32, tag="ps")
                    for kc in range(KC):
                        nc.tensor.matmul(
                            ps[0:1, :], h2_kT[:, kc:kc + 1],
                            wt[:, kc * 1024 + half * 512:
                               kc * 1024 + half * 512 + 512],
                            start=(kc == 0), stop=(kc == KC - 1))
                    wob_t = stagep.tile([1, 512], F32, tag="wob")
                    eng.dma_start(wob_t[:],
                                  AP(wob_d, t * 512, [[1, 1], [1, 512]]))
                    stage = stagep.tile([1, 512], F32, tag="stage")
                    nc.vector.tensor_add(stage[:], ps[0:1, :], wob_t[:])
                    o_ap = o_shard[:]
                    eng.dma_start(
                        AP(o_ap.tensor, o_ap.offset + t * 512,
                           [[1, 1], [1, 512]]),
                        stage[:],
                    )

            if stream_only:
                return
            # ---- AllGather the full o' ---------------------------------------
            o_full = dram.tile([OTOT], F32, tag="ofull")
            nc.gpsimd.collective_compute(
                "AllGather",
                mybir.AluOpType.bypass,
                ins=[o_shard[:].opt()],
                outs=[o_full[:].opt()],
                replica_groups=[list(range(NCORES))],
            )
            of_ap = o_full[:]

            # ---- diag: d[p, c] = exp(o'[128c + p]) ---------------------------
            d_raw = persist.tile([128, 4], F32, tag="draw")
            nc.gpsimd.dma_start(
                d_raw[:], AP(of_ap.tensor, of_ap.offset, [[1, 128], [128, 4]])
            )
            d_t = persist.tile([128, 4], F32, tag="dexp")
            nc.scalar.activation(d_t[:], d_raw[:],
                                 mybir.ActivationFunctionType.Exp)

            # ---- gather V = L'^T as [128, 4, 512] ----------------------------
            lt = persist.tile([128, 4, 512], F32, tag="lt")
            nc.gpsimd.dma_gather(
                lt[:],
                AP(of_ap.tensor, of_ap.offset, [[64, NWIN], [1, 512]]),
                gidx_sb[:],
                N,
                N,
                512,
                elem_step=64,
            )

            if debug_taps:
                nc.sync.dma_start(dbg_of[:], o_full[:])
                nc.sync.dma_start(dbg_lt[:], lt[:].rearrange("p a b -> p (a b)"))
                nc.sync.dma_start(dbg_h2[:], h2_kT[:])

            # ---- mask junk + insert exp-diag ---------------------------------
            tmp = persist.tile([128, 512], F32, tag="masktmp")
            for c in range(4):
                ltc = lt[:, c, :]
                nc.vector.tensor_mul(tmp[:], ltc, ltm[c][:])
                nc.vector.scalar_tensor_tensor(
                    ltc, eqm[c][:], d_t[:, c:c + 1], tmp[:],
                    mybir.AluOpType.mult, mybir.AluOpType.add,
                )
            if debug_taps:
                nc.sync.dma_start(dbg_ltm[:],
                                  lt[:].rearrange("p a b -> p (a b)"))

            # ---- D' = V^T V, written out flipped -----------------------------
            for m in range(4):
                psd = psum.tile([128, 512], F32, tag="ps")
                for c in range(4):
                    nc.tensor.matmul(psd[:], lt[:, c, m * 128:(m + 1) * 128],
                                     lt[:, c, :],
                                     start=(c == 0), stop=(c == 3))
                dout = stagep.tile([128, 512], F32, tag="dout")
                nc.vector.tensor_copy(dout[:], psd[:])
                # device emits D'; host flips both axes (D = J D' J)
                nc.sync.dma_start(
                    AP(out_d, 128 * m * N, [[N, 128], [1, 512]]),
                    dout[:],
                )

          if loop_n > 1:
            assert stream_only and reps == 1
            with tc.For_i(0, loop_n, 1):
                _emit_body()
          else:
            for _rep in range(reps):
                _emit_body()

    nc.compile()
    _PROGRAM_CACHE[key] = nc
    return nc


def prep_inputs(input, W0, b0, W1, b1, W2, b2, Wo, bo):
    """Host-side input prep: bias folding + Wo permutation/padding/sharding."""
    x = np.asarray(input, np.float32)
    w0a = np.concatenate([np.asarray(W0, np.float32),
                          np.asarray(b0, np.float32)[None, :]], axis=0)
    w1a = np.concatenate([np.asarray(W1, np.float32),
                          np.asarray(b1, np.float32)[None, :]], axis=0)
    w2a = np.concatenate([np.asarray(W2, np.float32),
                          np.asarray(b2, np.float32)[None, :]], axis=0)
    Wo = np.asarray(Wo, np.float32)
    bo = np.asarray(bo, np.float32)

    valid = COLMAP >= 0
    wo_perm = np.zeros((HID, OTOT), dtype=np.float32)
    wo_perm[:, valid] = Wo[:, COLMAP[valid]]
    wob_perm = np.zeros((OTOT,), dtype=np.float32)
    wob_perm[valid] = bo[COLMAP[valid]]

    in_maps = []
    for c in range(NCORES):
        sl = slice(c * OSH, (c + 1) * OSH)
        in_maps.append({
            "x": x,
            "w0": w0a,
            "w1": w1a,
            "w2": w2a,
            "wo": np.ascontiguousarray(wo_perm[:, sl]),
            "wob": np.ascontiguousarray(wob_perm[sl]),
            "gidx": GIDX,
        })
    return in_maps


def kernel(**inputs) -> np.ndarray:
    nc = build_program()
    in_maps = prep_inputs(**inputs)
    res = bass_utils.run_bass_kernel_spmd(nc, in_maps, list(range(NCORES)))
    dprime = res.results[0]["out"]
    return np.ascontiguousarray(dprime[::-1, ::-1]).reshape(1, N, N)


if __name__ == "__main__":
    # quick host-side check of the layout math against a numpy reference
    rng = np.random.default_rng(0)
    o = rng.standard_normal(OUT).astype(np.float32)
    # reference L
    L = np.zeros((N, N), np.float32)
    r, c = np.tril_indices(N, k=-1)
    L[r, c] = o[N:]
    L[np.arange(N), np.arange(N)] = np.exp(o[:N])
    D_ref = L @ L.T
    # o' = o[COLMAP] with zeros at padding
    op = np.zeros(OTOT, np.float32)
    op[COLMAP >= 0] = o[COLMAP[COLMAP >= 0]]
    # gather sim
    V = np.zeros((N, N), np.float32)
    idx = GIDX  # wrapped
    for g in range(N):
        w = int(idx[g % 16, g // 16]) * 64
        V[g, :] = op[w:w + 512]
    # masks
    col = np.arange(N)[None, :]
    row = np.arange(N)[:, None]
    V = V * (col < row)
    V = V + (col == row) * np.exp(op[:N])[:, None]
    Dp = V.T @ V
    D = Dp[::-1, ::-1]
    print("layout max err:", np.abs(D - D_ref).max(),
          "scale:", np.abs(D_ref).max())



# revision 5
# speedup vs baseline: 1.7141x; 1.7141x over previous
"""Trainium2 Bass kernel for nn_Damping: MLP trunk -> huge output layer ->
tril scatter -> D = L @ L.T, distributed over 8 NeuronCores.

Strategy (tensor-parallel, collective-free):
  - The dominant cost is streaming Wo [1024 x 131328] from HBM. Columns of L
    are distributed whole across the 8 cores (each core owns a set of
    "segments" = strict-lower columns of L, exactly 18368 elements each), so
    D_off = Loff @ Loff.T = sum_k col_k col_k^T is additive over cores: each
    core computes a [512,512] partial from ONLY its own shard and the host
    sums the 8 partials. No AllGather, no barrier.
  - The off-diag Wo shard is streamed in bf16 (half the HBM bytes, 1-pass PE
    matmuls instead of fp32's 2-pass LOW_HIGH). The 512 diagonal columns are
    computed replicated in f32, since exp(diag)^2 sets the output scale and
    dominates the rel-err metric.
  - The diag cross terms (Ld@Loff.T + Loff@Ld + Ld^2, O(n^2) ~ 0.5 MFLOP) are
    assembled on the host from the o-vector, which each core emits anyway
    (74KB). Device keeps all O(n*HID*OUT) + O(n^3) work.
  - Per-core scatter stays the flipped-window dma_gather trick of the
    baseline: each owned segment g holds column 511-g of L bottom-up at a
    64-aligned offset, so V-row g is one 512-wide gather window + keep-mask.
    Row->window indices and keep-masks are per-core INPUT data, keeping the
    program SPMD-identical across cores.
"""

import sys

sys.path.insert(0, "/opt/trn_rl_repo")

import numpy as np

import concourse.bass as bass
import concourse.bacc as bacc
import concourse.mybir as mybir
import concourse.tile as tile
from concourse.ap import AP
from concourse import bass_utils

N = 512
HID = 1024
OUT = N + N * (N - 1) // 2  # 131328
NCORES = 8
KC = HID // 128  # 8 k-chunks of the 1024-dim contraction

OSH = 18432        # per-core shard width (36 psum tiles of 512)
OSH_DATA = 18368   # real elements per core (exact 1/8 of off-diag total)
OBUF = OSH + 448   # shard buffer incl. gather-window overrun pad
NWIN = (OBUF - 512) // 64 + 1  # 288 gather windows
NT = OSH // 512    # 36 psum tiles per core
NCH = 2            # V-row chunks (max 159 rows per core -> 2x128 slots)
NIDX = NCH * 128   # 256 gather rows per core

F32 = mybir.dt.float32
BF16 = mybir.dt.bfloat16
I16 = mybir.dt.int16
NP_BF16 = mybir.dt.np(mybir.dt.bfloat16)


def _pack_segments():
    """Assign off-diag segments g=1..511 (size 64*ceil(g/64)) to 8 cores,
    exactly 18368 elements each (greedy largest-first exact fill)."""
    units = {g: (g + 63) // 64 for g in range(1, N)}
    pool = dict(units)
    cores = []
    for _ in range(NCORES):
        cap = OSH_DATA // 64
        mine = []
        for g in sorted(pool, key=lambda g: (-pool[g], g)):
            if pool[g] <= cap:
                mine.append(g)
                cap -= pool[g]
                del pool[g]
            if cap == 0:
                break
        assert cap == 0
        cores.append(sorted(mine))
    assert not pool
    assert all(len(m) <= NIDX for m in cores)
    return cores


CORE_SEGS = _pack_segments()


def _core_layout(segs):
    """Per-core layout: colmap (shard col -> o index or -1), gather indices
    (wrapped int16 [128, NIDX//16]), and keep-mask [128, NCH*512]."""
    colmap = np.full(OSH, -1, dtype=np.int64)
    idx = np.zeros(NIDX, dtype=np.int64)
    mask = np.zeros((128, NCH, 512), dtype=np.float32)
    pos = 0
    for slot, g in enumerate(segs):
        i = np.arange(g)
        r = (N - 1) - i          # L row index, bottom-up
        c = (N - 1) - g          # L col index
        colmap[pos + i] = N + r * (r - 1) // 2 + c
        idx[slot] = pos // 64
        mask[slot % 128, slot // 128, :g] = 1.0
        pos += 64 * ((g + 63) // 64)
    assert pos == OSH_DATA
    wrapped = np.zeros((16, NIDX // 16), dtype=np.int16)
    for s in range(NIDX):
        wrapped[s % 16, s // 16] = idx[s]
    gidx = np.tile(wrapped, (8, 1))  # replicated across gpsimd core groups
    return colmap, gidx, mask.reshape(128, NCH * 512)


CORE_LAYOUTS = [_core_layout(segs) for segs in CORE_SEGS]

_PROGRAM_CACHE = {}


def build_program():
    key = "nc"
    if key in _PROGRAM_CACHE:
        return _PROGRAM_CACHE[key]

    nc = bacc.Bacc("TRN2", target_bir_lowering=False, debug=False,
                   num_devices=NCORES)

    x_d = nc.dram_tensor("x", [N], F32, kind="ExternalInput")
    w0_d = nc.dram_tensor("w0", [4 * 128 + 1, HID], F32, kind="ExternalInput")
    w1_d = nc.dram_tensor("w1", [KC * 128 + 1, HID], F32, kind="ExternalInput")
    w2_d = nc.dram_tensor("w2", [KC * 128 + 1, HID], F32, kind="ExternalInput")
    wdiag_d = nc.dram_tensor("wdiag", [KC * 128 + 1, N], F32,
                             kind="ExternalInput")
    wo_d = nc.dram_tensor("wo", [HID, OSH], BF16, kind="ExternalInput")
    wob_d = nc.dram_tensor("wob", [OSH], F32, kind="ExternalInput")
    gidx_d = nc.dram_tensor("gidx", [128, NIDX // 16], I16,
                            kind="ExternalInput")
    mask_d = nc.dram_tensor("mask", [128, NCH * 512], F32,
                            kind="ExternalInput")
    out_d = nc.dram_tensor("out", [N, N], F32, kind="ExternalOutput")
    oshard_d = nc.dram_tensor("oshard", [OSH], F32, kind="ExternalOutput")
    odiag_d = nc.dram_tensor("odiag", [N], F32, kind="ExternalOutput")

    with tile.TileContext(nc) as tc:
        with (
            tc.tile_pool(name="wop", bufs=4) as wop,
            tc.tile_pool(name="trunkp", bufs=4) as trunkp,
            tc.tile_pool(name="persist", bufs=1) as persist,
            tc.tile_pool(name="stagep", bufs=4) as stagep,
            tc.tile_pool(name="psum", bufs=6, space="PSUM") as psum,
            tc.tile_pool(name="dram", bufs=2, space="DRAM") as dram,
        ):
            # ---- per-core metadata loads --------------------------------
            gidx_sb = persist.tile([128, NIDX // 16], I16, tag="gidx")
            nc.gpsimd.dma_start(gidx_sb[:], gidx_d[:])
            mask_sb = persist.tile([128, NCH * 512], F32, tag="mask")
            nc.sync.dma_start(mask_sb[:], mask_d[:])


            # ---- trunk: x -> h2 ----------------------------------------
            def to_kT(src_ap_flat, n_elems, tag, dt):
                """DRAM [n_elems] -> SBUF [128, n_elems//128 + 1] k-chunk
                layout with a trailing [1,0,..] column for the bias matmul."""
                ncols = n_elems // 128
                hk = persist.tile([128, ncols + 1], dt, tag=tag)
                nc.vector.memset(hk[:, ncols:ncols + 1], 0.0)
                nc.vector.memset(hk[0:1, ncols:ncols + 1], 1.0)
                # fine-strided pattern: HWDGE wedges on it, SWDGE handles it
                nc.gpsimd.dma_start(
                    hk[:, 0:ncols],
                    AP(src_ap_flat.tensor, src_ap_flat.offset,
                       [[1, 128], [128, ncols]]),
                )
                return hk

            x_kT = to_kT(x_d.ap(), N, "xkT", F32)

            def trunk_layer(h_kT, w_dram, kchunks, tag, also_bf16=False):
                wb = stagep.tile([1, HID], F32, tag="wbias")
                nc.sync.dma_start(wb[:],
                                  w_dram[kchunks * 128:kchunks * 128 + 1, :])
                h_sb = persist.tile([1, HID], F32, tag=f"h_sb_{tag}")
                h_sb_bf = None
                if also_bf16:
                    h_sb_bf = persist.tile([1, HID], BF16, tag=f"h_bf_{tag}")
                for nh in range(2):
                    ps = psum.tile([128, 512], F32, tag="ps")
                    for kcc in range(kchunks):
                        wt = trunkp.tile([128, 512], F32, tag="wt")
                        nc.sync.dma_start(
                            wt[:],
                            w_dram[kcc * 128:(kcc + 1) * 128,
                                   nh * 512:(nh + 1) * 512],
                        )
                        nc.tensor.matmul(ps[0:1, :], h_kT[:, kcc:kcc + 1],
                                         wt[:], start=(kcc == 0), stop=False)
                    nc.tensor.matmul(ps[0:1, :],
                                     h_kT[0:1, kchunks:kchunks + 1],
                                     wb[0:1, nh * 512:(nh + 1) * 512],
                                     start=False, stop=True)
                    nc.scalar.activation(h_sb[0:1, nh * 512:(nh + 1) * 512],
                                         ps[0:1, :],
                                         mybir.ActivationFunctionType.Tanh)
                    if also_bf16:
                        nc.scalar.activation(
                            h_sb_bf[0:1, nh * 512:(nh + 1) * 512],
                            ps[0:1, :], mybir.ActivationFunctionType.Tanh)
                # bounce through DRAM to re-layout [1,HID] -> [128, KC+1]
                hd = dram.tile([HID], F32, tag=f"hdram_{tag}")
                nc.sync.dma_start(hd[:], h_sb[0:1, :])
                out = to_kT(hd[:], HID, f"kT_{tag}", F32)
                out_bf = None
                if also_bf16:
                    hdb = dram.tile([HID], BF16, tag=f"hdramb_{tag}")
                    nc.scalar.dma_start(hdb[:], h_sb_bf[0:1, :])
                    out_bf = to_kT(hdb[:], HID, f"kTb_{tag}", BF16)
                return out, out_bf

            h0_kT, _ = trunk_layer(x_kT, w0_d, 4, "l0")
            h1_kT, _ = trunk_layer(h0_kT, w1_d, KC, "l1")
            h2_kT, h2b_kT = trunk_layer(h1_kT, w2_d, KC, "l2", also_bf16=True)

            # ---- diag head: o_diag = h2 @ Wo[:, :512] + bo[:512] (f32) --
            wdb = stagep.tile([1, N], F32, tag="wdb")
            nc.sync.dma_start(wdb[:], wdiag_d[KC * 128:KC * 128 + 1, :])
            psd0 = psum.tile([128, 512], F32, tag="ps")
            for kcc in range(KC):
                wdt = trunkp.tile([128, 512], F32, tag="wt")
                nc.sync.dma_start(wdt[:], wdiag_d[kcc * 128:(kcc + 1) * 128, :])
                nc.tensor.matmul(psd0[0:1, :], h2_kT[:, kcc:kcc + 1], wdt[:],
                                 start=(kcc == 0), stop=False)
            nc.tensor.matmul(psd0[0:1, :], h2_kT[0:1, KC:KC + 1], wdb[0:1, :],
                             start=False, stop=True)
            dstage = stagep.tile([1, N], F32, tag="dstage")
            nc.vector.tensor_copy(dstage[:], psd0[0:1, :])
            nc.sync.dma_start(odiag_d[:], dstage[:])

            # ---- o' shard buffer (+ zeroed gather-overrun pad) ----------
            o_buf = dram.tile([OBUF], F32, tag="obuf")
            zpad = stagep.tile([1, 448], F32, tag="zpad")
            nc.vector.memset(zpad[:], 0.0)
            ob_ap = o_buf[:]
            nc.sync.dma_start(
                AP(ob_ap.tensor, ob_ap.offset + OSH, [[1, 1], [1, 448]]),
                zpad[:])

            # ---- off-diag stream: o'_shard = h2 @ Wo_shard + bias (bf16) -
            # Wo streamed as 2MB bf16 tiles (2 n-tiles of 512 x 8 k-chunks),
            # alternating between the two HWDGE rings so transfers overlap.
            for tt in range(NT // 2):
                eng = nc.sync if (tt % 2 == 0) else nc.scalar
                wt = wop.tile([128, 2 * KC * 512], BF16, tag="wo")
                eng.dma_start(
                    wt[:],
                    AP(wo_d, tt * 1024,
                       [[OSH, 128], [128 * OSH, KC], [1, 1024]]),
                )
                for half in range(2):
                    t = 2 * tt + half
                    ps = psum.tile([128, 512], F32, tag="ps")
                    for kcc in range(KC):
                        nc.tensor.matmul(
                            ps[0:1, :], h2b_kT[:, kcc:kcc + 1],
                            wt[:, kcc * 1024 + half * 512:
                               kcc * 1024 + half * 512 + 512],
                            start=(kcc == 0), stop=(kcc == KC - 1))
                    wob_t = stagep.tile([1, 512], F32, tag="wob")
                    eng.dma_start(wob_t[:],
                                  AP(wob_d, t * 512, [[1, 1], [1, 512]]))
                    stage = stagep.tile([1, 512], F32, tag="stage")
                    nc.vector.tensor_add(stage[:], ps[0:1, :], wob_t[:])
                    eng.dma_start(
                        AP(ob_ap.tensor, ob_ap.offset + t * 512,
                           [[1, 1], [1, 512]]),
                        stage[:],
                    )

            # host reads the raw o' shard for the diag cross terms
            nc.scalar.dma_start(
                oshard_d[:],
                AP(ob_ap.tensor, ob_ap.offset, [[1, 1], [1, OSH]]))

            # ---- gather V rows (this core's columns of L, flipped) ------
            lt = persist.tile([128, NCH, 512], F32, tag="lt")
            nc.gpsimd.dma_gather(
                lt[:],
                AP(ob_ap.tensor, ob_ap.offset, [[64, NWIN], [1, 512]]),
                gidx_sb[:],
                NIDX,
                NIDX,
                512,
                elem_step=64,
            )

            # ---- mask junk + cast to bf16 -------------------------------
            ltb = persist.tile([128, NCH, 512], BF16, tag="ltb")
            for ch in range(NCH):
                nc.vector.tensor_mul(ltb[:, ch, :], lt[:, ch, :],
                                     mask_sb[:, ch * 512:(ch + 1) * 512])

            # ---- partial D' = V^T V, written out in flipped frame -------
            for m in range(4):
                psd = psum.tile([128, 512], F32, tag="ps")
                for ch in range(NCH):
                    nc.tensor.matmul(psd[:], ltb[:, ch, m * 128:(m + 1) * 128],
                                     ltb[:, ch, :],
                                     start=(ch == 0), stop=(ch == NCH - 1))
                dout = stagep.tile([128, 512], F32, tag="dout")
                nc.vector.tensor_copy(dout[:], psd[:])
                nc.sync.dma_start(
                    AP(out_d, 128 * m * N, [[N, 128], [1, 512]]),
                    dout[:],
                )

    nc.compile()
    _PROGRAM_CACHE[key] = nc
    return nc


def prep_inputs(input, W0, b0, W1, b1, W2, b2, Wo, bo):
    """Host-side input prep: bias folding + per-core Wo permutation."""
    x = np.asarray(input, np.float32)
    w0a = np.concatenate([np.asarray(W0, np.float32),
                          np.asarray(b0, np.float32)[None, :]], axis=0)
    w1a = np.concatenate([np.asarray(W1, np.float32),
                          np.asarray(b1, np.float32)[None, :]], axis=0)
    w2a = np.concatenate([np.asarray(W2, np.float32),
                          np.asarray(b2, np.float32)[None, :]], axis=0)
    Wo = np.asarray(Wo, np.float32)
    bo = np.asarray(bo, np.float32)
    wdiag = np.concatenate([Wo[:, :N], bo[:N][None, :]], axis=0)

    in_maps = []
    for c in range(NCORES):
        colmap, gidx, mask = CORE_LAYOUTS[c]
        valid = colmap >= 0
        wo_c = np.zeros((HID, OSH), dtype=NP_BF16)
        wo_c[:, valid] = Wo[:, colmap[valid]].astype(NP_BF16)
        wob_c = np.zeros((OSH,), dtype=np.float32)
        wob_c[valid] = bo[colmap[valid]]
        in_maps.append({
            "x": x,
            "w0": w0a,
            "w1": w1a,
            "w2": w2a,
            "wdiag": wdiag,
            "wo": wo_c,
            "wob": wob_c,
            "gidx": gidx,
            "mask": mask,
        })
    return in_maps


def assemble(results):
    """Host: sum partials, add diag cross terms, unflip."""
    dp = np.zeros((N, N), np.float64)
    o_full = np.zeros(OUT, np.float32)
    for c in range(NCORES):
        dp += results[c]["out"].astype(np.float64)
        colmap = CORE_LAYOUTS[c][0]
        valid = colmap >= 0
        o_full[colmap[valid]] = results[c]["oshard"][valid]
    o_full[:N] = results[0]["odiag"]
    d_off = dp[::-1, ::-1]  # unflip: D_off = J (V^T V) J

    dvec = np.exp(o_full[:N].astype(np.float64))
    loff = np.zeros((N, N), np.float64)
    r, c = np.tril_indices(N, k=-1)
    loff[r, c] = o_full[N:]
    cross = loff * dvec[None, :]  # Loff @ diag(d)
    D = d_off + cross + cross.T + np.diag(dvec * dvec)
    return np.ascontiguousarray(D.astype(np.float32)).reshape(1, N, N)


def kernel(**inputs) -> np.ndarray:
    nc = build_program()
    in_maps = prep_inputs(**inputs)
    res = bass_utils.run_bass_kernel_spmd(nc, in_maps, list(range(NCORES)))
    return assemble(res.results)


if __name__ == "__main__":
    # host-side selftest of the layout/partial math against numpy
    rng = np.random.default_rng(0)
    o = rng.standard_normal(OUT).astype(np.float32) * 0.1
    L = np.zeros((N, N), np.float32)
    r, c = np.tril_indices(N, k=-1)
    L[r, c] = o[N:]
    L[np.arange(N), np.arange(N)] = np.exp(o[:N])
    D_ref = L @ L.T

    # simulate the device: per-core shard, gather, mask, partial
    results = []
    for cid in range(NCORES):
        colmap, gidx, mask = CORE_LAYOUTS[cid]
        obuf = np.zeros(OBUF, np.float32)
        valid = colmap >= 0
        obuf[:OSH][valid] = o[colmap[valid]]
        V = np.zeros((NIDX, 512), np.float32)
        for s in range(NIDX):
            w = int(gidx[s % 16, s // 16]) * 64
            V[s, :] = obuf[w:w + 512]
        V = V * mask.reshape(128, NCH, 512).transpose(1, 0, 2).reshape(NIDX, 512)
        Vb = V.astype(NP_BF16).astype(np.float32)
        results.append({
            "out": (Vb.T @ Vb).astype(np.float32),
            "oshard": obuf[:OSH].copy(),
            "odiag": o[:N].copy(),
        })
    D = assemble(results)[0]
    print("layout max err:", np.abs(D - D_ref).max(),
          "scale:", np.abs(D_ref).max())


# revision 8
# speedup vs baseline: 2.4401x; 1.4235x over previous
"""Trainium2 Bass kernel for nn_Damping: MLP trunk -> huge output layer ->
tril scatter -> D = L @ L.T, distributed over 8 NeuronCores.

Strategy (tensor-parallel, collective-free):
  - The dominant cost is streaming Wo [1024 x 131328] from HBM. Columns of L
    are distributed whole across the 8 cores, so D_off = Loff @ Loff.T =
    sum_k col_k col_k^T is additive over cores: each core computes a
    [512,512] partial from ONLY its own shard and the host sums the 8
    partials. No AllGather, no barrier.
  - The off-diag Wo shard is streamed in bf16 (half the HBM bytes, 1-pass PE
    matmuls instead of fp32's 2-pass LOW_HIGH), host-prepacked so every DMA
    line is 16KB contiguous. The 512 diagonal columns are computed
    replicated in f32, since exp(diag)^2 sets the output scale and
    dominates the rel-err metric.
  - Segment->core assignment is by SIZE CLASS: strict-lower column g of L
    has 64*ceil(g/64) slots; each core gets exactly 8 segments of each
    class u (class 8: one core gets 7 + a zero dummy). The per-core shard
    layout is therefore compile-time-fixed: class u at offset 512*u*(u-1)/2,
    8 windows of stride 64u. The V-"gather" becomes 8 affine HWDGE DMAs --
    no gpsimd dma_gather, no 16-20us SWDGE dge_drain in the tail. Which g
    each window holds varies per core, but only the keep-MASK (input data)
    depends on that, keeping the program SPMD-identical.
  - The diag cross terms (Ld@Loff.T + ..., O(n^2) ~ 0.5 MFLOP) are assembled
    on the host from the o-vector, which the cores emit anyway (74KB).
    Device keeps all O(HID*OUT) + O(n^3) work.
  - Trunk runs in bf16 (weights preloaded whole into SBUF over the vector
    ring so they never queue behind the Wo stream), with the last tanh also
    written in f32 (from the f32 PSUM) to feed the f32 diag head.
"""

import sys

sys.path.insert(0, "/opt/trn_rl_repo")

import numpy as np

import concourse.bass as bass
import concourse.bacc as bacc
import concourse.mybir as mybir
import concourse.tile as tile
from concourse.ap import AP
from concourse import bass_utils

N = 512
HID = 1024
OUT = N + N * (N - 1) // 2  # 131328
NCORES = 8
KC = HID // 128  # 8 k-chunks of the 1024-dim contraction

OSH = 18432        # per-core shard width (36 psum tiles of 512)
NT = OSH // 512    # 36 psum tiles per core
NTT = NT // 2      # 18 2MB wo stream tiles
SPC = 8            # segments per size class per core
POS = [512 * u * (u - 1) // 2 for u in range(9)]  # class-u offset in shard

F32 = mybir.dt.float32
BF16 = mybir.dt.bfloat16
NP_BF16 = mybir.dt.np(mybir.dt.bfloat16)


def _assign_segments():
    """Per-core segment lists: core c gets segments [c::8] of each size
    class u (g in (64(u-1), 64u]); class 8 is short one -> core 7 gets a
    zero-width dummy (g=0)."""
    cores = [[] for _ in range(NCORES)]
    for u in range(1, 9):
        gs = [g for g in range(1, N) if (g + 63) // 64 == u]
        if len(gs) < NCORES * SPC:
            gs = gs + [0] * (NCORES * SPC - len(gs))  # dummies
        for c in range(NCORES):
            cores[c].extend(gs[c::NCORES])
    return cores


CORE_SEGS = _assign_segments()


def _core_layout(segs):
    """colmap (shard col -> o index or -1) and keep-mask [128, 512].
    Slot s = 8*(u-1)+i holds the i-th class-u segment at partition s."""
    colmap = np.full(OSH, -1, dtype=np.int64)
    mask = np.zeros((128, 512), dtype=np.float32)
    for s, g in enumerate(segs):
        u = s // SPC + 1
        i = s % SPC
        pos = POS[u] + i * 64 * u
        if g == 0:
            continue
        assert (g + 63) // 64 == u
        idx = np.arange(g)
        r = (N - 1) - idx        # L row index, bottom-up
        cc = (N - 1) - g         # L col index
        colmap[pos + idx] = N + r * (r - 1) // 2 + cc
        mask[s, :g] = 1.0
    return colmap, mask


CORE_LAYOUTS = [_core_layout(segs) for segs in CORE_SEGS]

_PROGRAM_CACHE = {}


def build_program():
    key = "nc"
    if key in _PROGRAM_CACHE:
        return _PROGRAM_CACHE[key]

    nc = bacc.Bacc("TRN2", target_bir_lowering=False, debug=False,
                   num_devices=NCORES)

    # trunk weights host-prepacked to [128, KC*1024] bf16 (16KB DMA lines);
    # biases as separate bf16 rows. wo prepacked per 2MB stream tile.
    xb_d = nc.dram_tensor("xb", [N], BF16, kind="ExternalInput")
    w0_d = nc.dram_tensor("w0", [128, 4 * HID], BF16, kind="ExternalInput")
    w1_d = nc.dram_tensor("w1", [128, KC * HID], BF16, kind="ExternalInput")
    w2_d = nc.dram_tensor("w2", [128, KC * HID], BF16, kind="ExternalInput")
    wb_d = nc.dram_tensor("wb", [3, HID], BF16, kind="ExternalInput")
    wdiag_d = nc.dram_tensor("wdiag", [128, KC * N], F32,
                             kind="ExternalInput")
    bdiag_d = nc.dram_tensor("bdiag", [N], F32, kind="ExternalInput")
    wo_d = nc.dram_tensor("wo", [NTT, 128, 2 * KC * 512], BF16,
                          kind="ExternalInput")
    wob_d = nc.dram_tensor("wob", [OSH], F32, kind="ExternalInput")
    mask_d = nc.dram_tensor("mask", [128, 512], F32, kind="ExternalInput")
    out_d = nc.dram_tensor("out", [N, N], F32, kind="ExternalOutput")
    oshard_d = nc.dram_tensor("oshard", [OSH], F32, kind="ExternalOutput")
    odiag_d = nc.dram_tensor("odiag", [N], F32, kind="ExternalOutput")

    with tile.TileContext(nc) as tc:
        with (
            tc.tile_pool(name="wop", bufs=4) as wop,
            tc.tile_pool(name="persist", bufs=1) as persist,
            tc.tile_pool(name="stagep", bufs=6) as stagep,
            tc.tile_pool(name="psum", bufs=6, space="PSUM") as psum,
            tc.tile_pool(name="dram", bufs=2, space="DRAM") as dram,
        ):
            # ---- persistent loads (vector ring: off the Wo stream rings) -
            mask_sb = persist.tile([128, 512], F32, tag="mask")
            nc.sync.dma_start(mask_sb[:], mask_d[:])
            w0_sb = persist.tile([128, 4 * HID], BF16, tag="w0")
            nc.sync.dma_start(w0_sb[:], w0_d[:])
            w1_sb = persist.tile([128, KC * HID], BF16, tag="w1")
            nc.sync.dma_start(w1_sb[:], w1_d[:])
            w2_sb = persist.tile([128, KC * HID], BF16, tag="w2")
            nc.scalar.dma_start(w2_sb[:], w2_d[:])
            wb_rows = []
            for li in range(3):
                wbr = persist.tile([1, HID], BF16, tag=f"wb{li}")
                nc.scalar.dma_start(wbr[:], wb_d[li:li + 1, :])
                wb_rows.append(wbr)
            wdiag_sb = persist.tile([128, KC * N], F32, tag="wdiag")
            nc.scalar.dma_start(wdiag_sb[:], wdiag_d[:])
            bdiag_sb = persist.tile([1, N], F32, tag="bdiag")
            nc.scalar.dma_start(bdiag_sb[:], bdiag_d[0:N])

            # ---- trunk: x -> h2 (bf16 weights/activations, f32 psum) -----
            def to_kT(src_ap_flat, n_elems, tag, dt):
                """DRAM [n_elems] -> SBUF [128, n_elems//128 + 1] k-chunk
                layout with a trailing [1,0,..] column for the bias matmul."""
                ncols = n_elems // 128
                hk = persist.tile([128, ncols + 1], dt, tag=tag)
                nc.vector.memset(hk[:, ncols:ncols + 1], 0.0)
                nc.vector.memset(hk[0:1, ncols:ncols + 1], 1.0)
                # fine-strided pattern: HWDGE wedges on it, SWDGE handles it
                nc.gpsimd.dma_start(
                    hk[:, 0:ncols],
                    AP(src_ap_flat.tensor, src_ap_flat.offset,
                       [[1, 128], [128, ncols]]),
                )
                return hk

            x_kT = to_kT(xb_d.ap(), N, "xkT", BF16)

            def trunk_layer(h_kT, w_sb, bias_row, kchunks, tag,
                            also_f32=False):
                h_bf = persist.tile([1, HID], BF16, tag=f"hbf_{tag}")
                h_f32 = None
                if also_f32:
                    h_f32 = persist.tile([1, HID], F32, tag=f"hf_{tag}")
                for nh in range(2):
                    ps = psum.tile([128, 512], F32, tag="ps")
                    for kcc in range(kchunks):
                        nc.tensor.matmul(
                            ps[0:1, :], h_kT[:, kcc:kcc + 1],
                            w_sb[:, kcc * HID + nh * 512:
                                 kcc * HID + nh * 512 + 512],
                            start=(kcc == 0), stop=False)
                    nc.tensor.matmul(ps[0:1, :],
                                     h_kT[0:1, kchunks:kchunks + 1],
                                     bias_row[0:1, nh * 512:(nh + 1) * 512],
                                     start=False, stop=True)
                    nc.scalar.activation(h_bf[0:1, nh * 512:(nh + 1) * 512],
                                         ps[0:1, :],
                                         mybir.ActivationFunctionType.Tanh)
                    if also_f32:
                        nc.scalar.activation(
                            h_f32[0:1, nh * 512:(nh + 1) * 512],
                            ps[0:1, :], mybir.ActivationFunctionType.Tanh)
                # bounce through DRAM to re-layout [1,HID] -> [128, KC+1]
                hd = dram.tile([HID], BF16, tag=f"hdram_{tag}")
                nc.scalar.dma_start(hd[:], h_bf[0:1, :])
                out = to_kT(hd[:], HID, f"kT_{tag}", BF16)
                out_f32 = None
                if also_f32:
                    hdf = dram.tile([HID], F32, tag=f"hdramf_{tag}")
                    nc.scalar.dma_start(hdf[:], h_f32[0:1, :])
                    out_f32 = to_kT(hdf[:], HID, f"kTf_{tag}", F32)
                return out, out_f32

            h0_kT, _ = trunk_layer(x_kT, w0_sb, wb_rows[0][:], 4, "l0")
            h1_kT, _ = trunk_layer(h0_kT, w1_sb, wb_rows[1][:], KC, "l1")
            h2b_kT, h2f_kT = trunk_layer(h1_kT, w2_sb, wb_rows[2][:], KC,
                                         "l2", also_f32=True)

            # ---- diag head: o_diag = h2 @ Wo[:, :512] + bo[:512] (f32) --
            psd0 = psum.tile([128, 512], F32, tag="ps")
            for kcc in range(KC):
                nc.tensor.matmul(psd0[0:1, :], h2f_kT[:, kcc:kcc + 1],
                                 wdiag_sb[:, kcc * N:(kcc + 1) * N],
                                 start=(kcc == 0), stop=False)
            nc.tensor.matmul(psd0[0:1, :], h2f_kT[0:1, KC:KC + 1],
                             bdiag_sb[0:1, :], start=False, stop=True)
            dstage = stagep.tile([1, N], F32, tag="dstage")
            nc.vector.tensor_copy(dstage[:], psd0[0:1, :])
            nc.sync.dma_start(odiag_d[:], dstage[:])

            # ---- off-diag stream: o'_shard = h2 @ Wo_shard + bias (bf16) -
            o_buf = dram.tile([OSH], F32, tag="obuf")
            ob_ap = o_buf[:]
            rings = [nc.sync, nc.scalar]
            for tt in range(NTT):
                eng = rings[tt % 2]
                wt = wop.tile([128, 2 * KC * 512], BF16, tag="wo")
                eng.dma_start(
                    wt[:],
                    AP(wo_d, tt * 128 * 2 * KC * 512,
                       [[2 * KC * 512, 128], [1, 2 * KC * 512]]),
                )
                for half in range(2):
                    t = 2 * tt + half
                    ps = psum.tile([128, 512], F32, tag="ps")
                    for kcc in range(KC):
                        nc.tensor.matmul(
                            ps[0:1, :], h2b_kT[:, kcc:kcc + 1],
                            wt[:, kcc * 1024 + half * 512:
                               kcc * 1024 + half * 512 + 512],
                            start=(kcc == 0), stop=(kcc == KC - 1))
                    wob_t = stagep.tile([1, 512], F32, tag="wob")
                    eng.dma_start(wob_t[:],
                                  AP(wob_d, t * 512, [[1, 1], [1, 512]]))
                    stage = stagep.tile([1, 512], F32, tag="stage")
                    nc.vector.tensor_add(stage[:], ps[0:1, :], wob_t[:])
                    eng.dma_start(
                        AP(ob_ap.tensor, ob_ap.offset + t * 512,
                           [[1, 1], [1, 512]]),
                        stage[:],
                    )

            # host reads the raw o' shard for the diag cross terms
            nc.scalar.dma_start(
                oshard_d[:],
                AP(ob_ap.tensor, ob_ap.offset, [[1, 1], [1, OSH]]))

            # ---- load V rows: 8 affine window DMAs (one per size class) --
            lt = persist.tile([128, 512], F32, tag="lt")
            nc.vector.memset(lt[64:128, :], 0.0)
            for u in range(1, 9):
                eng = rings[u % 2]
                eng.dma_start(
                    lt[SPC * (u - 1):SPC * u, :],
                    AP(ob_ap.tensor, ob_ap.offset + POS[u],
                       [[64 * u, SPC], [1, 512]]),
                )

            # ---- mask junk + cast to bf16 -------------------------------
            ltb = persist.tile([128, 512], BF16, tag="ltb")
            nc.vector.tensor_mul(ltb[:], lt[:], mask_sb[:])

            # ---- partial D' = V^T V, written out in flipped frame -------
            for m in range(4):
                psd = psum.tile([128, 512], F32, tag="ps")
                nc.tensor.matmul(psd[:], ltb[:, m * 128:(m + 1) * 128],
                                 ltb[:], start=True, stop=True)
                dout = stagep.tile([128, 512], F32, tag="dout")
                nc.vector.tensor_copy(dout[:], psd[:])
                nc.sync.dma_start(
                    AP(out_d, 128 * m * N, [[N, 128], [1, 512]]),
                    dout[:],
                )

    nc.compile()
    _PROGRAM_CACHE[key] = nc
    return nc


def _pack_kT(w, kchunks):
    """[K, M] f32 -> [128, kchunks*M] bf16, row p holding all k-chunks'
    row (kc*128+p) contiguously (16KB DMA lines)."""
    K, M = w.shape
    assert K == kchunks * 128
    return np.ascontiguousarray(
        w.reshape(kchunks, 128, M).transpose(1, 0, 2).reshape(128, kchunks * M)
    )


def prep_inputs(input, W0, b0, W1, b1, W2, b2, Wo, bo):
    """Host-side input prep: bf16 casts, prepacking, per-core Wo permute."""
    x = np.asarray(input, np.float32)
    W0 = np.asarray(W0, np.float32)
    W1 = np.asarray(W1, np.float32)
    W2 = np.asarray(W2, np.float32)
    Wo = np.asarray(Wo, np.float32)
    bo = np.asarray(bo, np.float32)
    w0p = _pack_kT(W0, 4).astype(NP_BF16)
    w1p = _pack_kT(W1, KC).astype(NP_BF16)
    w2p = _pack_kT(W2, KC).astype(NP_BF16)
    wb = np.stack([np.asarray(b0, np.float32), np.asarray(b1, np.float32),
                   np.asarray(b2, np.float32)]).astype(NP_BF16)
    wdiag = _pack_kT(Wo[:, :N], KC)
    bdiag = bo[:N].copy()

    in_maps = []
    for c in range(NCORES):
        colmap, mask = CORE_LAYOUTS[c]
        valid = colmap >= 0
        wo_c = np.zeros((HID, OSH), dtype=np.float32)
        wo_c[:, valid] = Wo[:, colmap[valid]]
        # prepack: [NTT, 128, 8192]; wt[p, kc*1024+j] = wo_c[kc*128+p, tt*1024+j]
        wo_pack = np.ascontiguousarray(
            wo_c.reshape(KC, 128, NTT, 1024).transpose(2, 1, 0, 3)
            .reshape(NTT, 128, 2 * KC * 512)).astype(NP_BF16)
        wob_c = np.zeros((OSH,), dtype=np.float32)
        wob_c[valid] = bo[colmap[valid]]
        in_maps.append({
            "xb": x.astype(NP_BF16),
            "w0": w0p,
            "w1": w1p,
            "w2": w2p,
            "wb": wb,
            "wdiag": wdiag,
            "bdiag": bdiag,
            "wo": wo_pack,
            "wob": wob_c,
            "mask": mask,
        })
    return in_maps


def assemble(results):
    """Host: sum partials, add diag cross terms, unflip."""
    dp = np.zeros((N, N), np.float64)
    o_full = np.zeros(OUT, np.float32)
    for c in range(NCORES):
        dp += results[c]["out"].astype(np.float64)
        colmap = CORE_LAYOUTS[c][0]
        valid = colmap >= 0
        o_full[colmap[valid]] = results[c]["oshard"][valid]
    o_full[:N] = results[0]["odiag"]
    d_off = dp[::-1, ::-1]  # unflip: D_off = J (V^T V) J

    dvec = np.exp(o_full[:N].astype(np.float64))
    loff = np.zeros((N, N), np.float64)
    r, c = np.tril_indices(N, k=-1)
    loff[r, c] = o_full[N:]
    cross = loff * dvec[None, :]  # Loff @ diag(d)
    D = d_off + cross + cross.T + np.diag(dvec * dvec)
    return np.ascontiguousarray(D.astype(np.float32)).reshape(1, N, N)


def kernel(**inputs) -> np.ndarray:
    nc = build_program()
    in_maps = prep_inputs(**inputs)
    res = bass_utils.run_bass_kernel_spmd(nc, in_maps, list(range(NCORES)))
    return assemble(res.results)


if __name__ == "__main__":
    # host-side selftest of the layout/partial math against numpy
    rng = np.random.default_rng(0)
    o = rng.standard_normal(OUT).astype(np.float32) * 0.1
    L = np.zeros((N, N), np.float32)
    r, c = np.tril_indices(N, k=-1)
    L[r, c] = o[N:]
    L[np.arange(N), np.arange(N)] = np.exp(o[:N])
    D_ref = L @ L.T

    results = []
    for cid in range(NCORES):
        colmap, mask = CORE_LAYOUTS[cid]
        obuf = np.zeros(OSH, np.float32)
        valid = colmap >= 0
        obuf[valid] = o[colmap[valid]]
        V = np.zeros((128, 512), np.float32)
        for u in range(1, 9):
            for i in range(SPC):
                w = POS[u] + i * 64 * u
                V[SPC * (u - 1) + i, :] = obuf[w:w + 512]
        V = V * mask
        Vb = V.astype(NP_BF16).astype(np.float32)
        results.append({
            "out": (Vb.T @ Vb).astype(np.float32),
            "oshard": obuf.copy(),
            "odiag": o[:N].copy(),
        })
    D = assemble(results)[0]
    print("layout max err:", np.abs(D - D_ref).max(),
          "scale:", np.abs(D_ref).max())
